# revision 15
# baseline (speedup 1.0000x reference)
"""AttentiveRouter (MoE routing) Trainium2 Bass kernel.

Full inputs in, full outputs out. Data-parallel over tokens (B*S=8192)
across 8 NeuronCores, router weights replicated. The load-balance
reduction over expert_usage is finished on host from the gathered
expert_mask (64-element math).

Numerics: matmuls run as fp16 hi/lo split triples (W@x = Wh@xh + Wh@xl
+ Wl@xh) accumulated into one fp32 PSUM chain, giving ~2^-22 effective
input precision (fp32-grade logits; top-k indices must match the fp32
reference exactly — min rank2/rank3 logit gap on this data is 3e-5).
The PE honors fp16 subnormal inputs (verified on HW), so the lo parts
need no scaling.

Per-core pipeline (1024 tokens, 2 chunks of 512):
  - x tile -> PE fp32 transpose -> split from PSUM into xhT/xlT fp16
  - W1 k-tiles DMA'd fp32 (gpsimd+scalar queues), split on ACT (cast)
    + DVE (subtract) into W1h/W1l fp16; emission interleaved with the
    x stage so the engine streams don't serialize the startup
  - mm1 (hT form) per nb block: one PSUM chain accumulating 3 passes
    x 16 k tiles; Gelu+b1 fused into the ACT drain of the PSUM (bias
    is per-partition in hT layout); hT split into fp16 halves
  - mm2 in scoresT form (W2h/W2l stationary, hT halves moving) into a
    [64,512] PSUM chain; PE-stream-pipelined one nb behind mm1 so the
    hT drain latency is hidden; scoresT is PE-transposed back to
    token-partition scores [128,64]; b2 added during the PSUM->SBUF
    copy
  - epilogue per 128-token tile: softmax (exp with scale=1/T and
    bias=-rowmax/T from the temperature input, accum-out Z), top-2 via
    DVE max/max_index, normalized weights, one-hot scatter via
    iota+is_equal
"""

import numpy as np

import concourse.bass as bass
import concourse.tile as tile
from concourse import bacc, masks, mybir
from concourse.bass_utils import run_bass_kernel_spmd

NCORES = 8
B, S, D, E = 4, 2048, 2048, 64
TOKENS = B * S              # 8192
M_CORE = TOKENS // NCORES   # 1024 tokens per core
CHUNK = 512                 # tokens per mm1 chunk
NCHUNK = M_CORE // CHUNK    # 2
KT = D // 128               # 16 k tiles
NBT = D // 128              # 16 n blocks
TOP_K = 2
CAPACITY = int(1.25 * S)    # 2560 (static, matches reference)

FP = mybir.dt.float32
HP = mybir.dt.float16


def build_program():
    nc = bacc.Bacc("TRN2", target_bir_lowering=False, debug=False)

    x_d = nc.dram_tensor("x_shard", [M_CORE, D], FP, kind="ExternalInput").ap()
    w1_d = nc.dram_tensor("W1", [D, D], FP, kind="ExternalInput").ap()
    b1_d = nc.dram_tensor("b1", [D], FP, kind="ExternalInput").ap()
    w2_d = nc.dram_tensor("W2", [D, E], FP, kind="ExternalInput").ap()
    b2_d = nc.dram_tensor("b2", [E], FP, kind="ExternalInput").ap()
    t_d = nc.dram_tensor("temperature", [1], FP, kind="ExternalInput").ap()

    mask_d = nc.dram_tensor("mask_out", [M_CORE, E], FP, kind="ExternalOutput").ap()
    attn_d = nc.dram_tensor("attn_out", [M_CORE, E], FP, kind="ExternalOutput").ap()
    idx_d = nc.dram_tensor(
        "idx_out", [M_CORE, TOP_K], mybir.dt.int32, kind="ExternalOutput"
    ).ap()

    AF = mybir.ActivationFunctionType
    OP = mybir.AluOpType

    with tile.TileContext(nc) as tc:
        with (
            tc.tile_pool(name="consts", bufs=1) as consts,
            tc.tile_pool(name="w1p", bufs=1) as w1p,
            tc.tile_pool(name="w2p", bufs=1) as w2p,
            tc.tile_pool(name="stage", bufs=3) as stage,
            tc.tile_pool(name="xtp", bufs=1) as xtp,
            tc.tile_pool(name="htp", bufs=2) as htp,
            tc.tile_pool(name="sct", bufs=1) as sctp,
            tc.tile_pool(name="epi", bufs=2) as epi,
            tc.tile_pool(name="ps512", bufs=4, space="PSUM") as ps512,
            tc.tile_pool(name="psT", bufs=2, space="PSUM") as psT,
            tc.tile_pool(name="ps64", bufs=2, space="PSUM") as ps64,
        ):
            # ---- constants ----
            ident = consts.tile([128, 128], FP, tag="ident")
            masks.make_identity(nc, ident[:])

            iota_f = consts.tile([128, E], FP, tag="iota")
            nc.gpsimd.iota(
                iota_f[:], pattern=[[1, E]], base=0, channel_multiplier=0,
                allow_small_or_imprecise_dtypes=True,
            )

            ones1 = consts.tile([1, 128], FP, tag="ones1")
            nc.vector.memset(ones1[:], 1.0)

            b2_sb = consts.tile([1, E], FP, tag="b2sb")
            nc.sync.dma_start(b2_sb[:], b2_d[None, :])

            b1t = consts.tile([128, NBT], FP, tag="b1t")
            nc.sync.dma_start(b1t[:], b1_d.rearrange("(a b) -> b a", b=128))

            t_sb = consts.tile([1, 1], FP, tag="tsb")
            nc.sync.dma_start(t_sb[:], t_d[None, :])
            invt1 = consts.tile([1, 1], FP, tag="invt1")
            nc.vector.reciprocal(invt1[:], t_sb[:])
            ps_invt = ps64.tile([128, 1], FP, tag="ps64", name="ps_invt")
            nc.tensor.matmul(ps_invt[:], lhsT=ones1[:], rhs=invt1[:])
            invt_bc = consts.tile([128, 1], FP, tag="invtbc")
            nc.scalar.copy(invt_bc[:], ps_invt[:])
            ninvt_bc = consts.tile([128, 1], FP, tag="ninvtbc")
            nc.scalar.mul(ninvt_bc[:], ps_invt[:], -1.0)

            ps_b2 = ps64.tile([128, E], FP, tag="ps64", name="ps_b2")
            nc.tensor.matmul(ps_b2[:], lhsT=ones1[:], rhs=b2_sb[:])
            b2bc = consts.tile([128, E], FP, tag="b2bc")
            nc.scalar.copy(b2bc[:], ps_b2[:])

            # ---- W2 load + split (small) ----
            w2s = stage.tile([128, NBT * E], FP, tag="stage", name="w2s")
            for nb in range(NBT):
                nc.sync.dma_start(
                    w2s[:, nb * E:(nb + 1) * E], w2_d[nb * 128:(nb + 1) * 128, :]
                )
            w2h = w2p.tile([128, NBT * E], HP, tag="w2h")
            nc.scalar.copy(w2h[:], w2s[:])
            w2l = w2p.tile([128, NBT * E], HP, tag="w2l")
            nc.vector.scalar_tensor_tensor(
                w2l[:], in0=w2h[:], scalar=-1.0, in1=w2s[:],
                op0=OP.mult, op1=OP.add,
            )

            w1h = [None] * KT
            w1l = [None] * KT
            xh = [
                xtp.tile([128, CHUNK], HP, tag=f"xh_{k}", name=f"xh_{k}")
                for k in range(KT)
            ]
            xl = [
                xtp.tile([128, CHUNK], HP, tag=f"xl_{k}", name=f"xl_{k}")
                for k in range(KT)
            ]

            def emit_w1_split(k):
                ws = stage.tile([128, D], FP, tag="stage", name=f"w1s_{k}")
                eng = [nc.gpsimd, nc.scalar, nc.sync][k % 3]
                eng.dma_start(ws[:], w1_d[k * 128:(k + 1) * 128, :])
                wh = w1p.tile([128, D], HP, tag=f"w1h_{k}", name=f"w1h_{k}")
                nc.scalar.copy(wh[:], ws[:])
                wl = w1p.tile([128, D], HP, tag=f"w1l_{k}", name=f"w1l_{k}")
                nc.vector.scalar_tensor_tensor(
                    wl[:], in0=wh[:], scalar=-1.0, in1=ws[:],
                    op0=OP.mult, op1=OP.add,
                )
                w1h[k] = wh
                w1l[k] = wl

            def emit_x_stage(ch, ms):
                m0 = ch * CHUNK
                xs = stage.tile([128, D], FP, tag="stage", name=f"xs_{ch}_{ms}")
                nc.sync.dma_start(xs[:], x_d[m0 + ms * 128:m0 + (ms + 1) * 128, :])
                cols = slice(ms * 128, (ms + 1) * 128)
                for k in range(KT):
                    pt = ps512.tile([128, 128], FP, tag="ps512",
                                    name=f"pt_{ch}_{ms}_{k}")
                    nc.tensor.transpose(pt[:], xs[:, k * 128:(k + 1) * 128],
                                        ident[:])
                    nc.scalar.copy(xh[k][:, cols], pt[:])
                    nc.vector.scalar_tensor_tensor(
                        xl[k][:, cols], in0=xh[k][:, cols], scalar=-1.0,
                        in1=pt[:], op0=OP.mult, op1=OP.add,
                    )

            def emit_mm2(st, nb, hth, htl):
                ec = slice(nb * E, (nb + 1) * E)
                nc.tensor.matmul(st[:], lhsT=w2h[:, ec], rhs=hth[:],
                                 start=(nb == 0), stop=False)
                nc.tensor.matmul(st[:], lhsT=w2h[:, ec], rhs=htl[:],
                                 start=False, stop=False)
                nc.tensor.matmul(st[:], lhsT=w2l[:, ec], rhs=hth[:],
                                 start=False, stop=(nb == NBT - 1))

            # interleave W1 split with chunk-0 x staging so the ACT/DVE
            # streams alternate between them during the DMA-bound startup
            for k in range(KT):
                emit_w1_split(k)
                if k % 4 == 3:
                    emit_x_stage(0, k // 4)

            def emit_ht_split(ch, nb, acc):
                ht32 = htp.tile([128, CHUNK], FP, tag="ht32",
                                name=f"ht32_{ch}_{nb}")
                nc.scalar.activation(ht32[:], acc[:], AF.Gelu,
                                     bias=b1t[:, nb:nb + 1])
                hth = htp.tile([128, CHUNK], HP, tag="hth",
                               name=f"hth_{ch}_{nb}")
                nc.scalar.copy(hth[:], ht32[:])
                htl = htp.tile([128, CHUNK], HP, tag="htl",
                               name=f"htl_{ch}_{nb}")
                nc.vector.scalar_tensor_tensor(
                    htl[:], in0=hth[:], scalar=-1.0, in1=ht32[:],
                    op0=OP.mult, op1=OP.add,
                )
                return hth, htl

            for ch in range(NCHUNK):
                m0 = ch * CHUNK

                st = psT.tile([64, CHUNK], FP, tag="psT", name=f"st_{ch}")
                pending = None
                nb_start = 0
                if ch == 0:
                    # startup: k-major sweeps across 4 live PSUM chains so
                    # the PE has ~4x more ready work per arriving W1 k-tile
                    # (W1 split is DMA/ACT/DVE-paced here); hi passes first
                    # since W1h (ACT cast) lands before W1l (DVE subtract)
                    nb_start = 4
                    accs = [
                        ps512.tile([128, CHUNK], FP, tag="ps512",
                                   name=f"acc_{ch}_{nb}")
                        for nb in range(4)
                    ]
                    for k in range(KT):
                        for nb in range(4):
                            nbc = slice(nb * 128, (nb + 1) * 128)
                            nc.tensor.matmul(accs[nb][:], lhsT=w1h[k][:, nbc],
                                             rhs=xh[k][:],
                                             start=(k == 0), stop=False)
                    for k in range(KT):
                        for nb in range(4):
                            nbc = slice(nb * 128, (nb + 1) * 128)
                            nc.tensor.matmul(accs[nb][:], lhsT=w1h[k][:, nbc],
                                             rhs=xl[k][:],
                                             start=False, stop=False)
                            nc.tensor.matmul(accs[nb][:], lhsT=w1l[k][:, nbc],
                                             rhs=xh[k][:],
                                             start=False, stop=(k == KT - 1))
                    for nb in range(4):
                        if pending is not None:
                            emit_mm2(st, *pending)
                        hth, htl = emit_ht_split(ch, nb, accs[nb])
                        pending = (nb, hth, htl)

                for nb in range(nb_start, NBT):
                    acc = ps512.tile([128, CHUNK], FP, tag="ps512",
                                     name=f"acc_{ch}_{nb}")
                    nbc = slice(nb * 128, (nb + 1) * 128)
                    for k in range(KT):
                        nc.tensor.matmul(acc[:], lhsT=w1h[k][:, nbc],
                                         rhs=xh[k][:],
                                         start=(k == 0), stop=False)
                        nc.tensor.matmul(acc[:], lhsT=w1h[k][:, nbc],
                                         rhs=xl[k][:],
                                         start=False, stop=False)
                        nc.tensor.matmul(acc[:], lhsT=w1l[k][:, nbc],
                                         rhs=xh[k][:],
                                         start=False, stop=(k == KT - 1))
                    # PE pipeline: previous nb's mm2 lands after this nb's
                    # mm1 so its hT drain latency is hidden
                    if pending is not None:
                        emit_mm2(st, *pending)
                    hth, htl = emit_ht_split(ch, nb, acc)
                    pending = (nb, hth, htl)
                emit_mm2(st, *pending)

                # stage the NEXT chunk's x before this chunk's epilogue so
                # its PE transposes run while the DVE handles the scoresT
                # copy (the xh/xl WAR hazard is already clear: last mm1
                # read of this chunk just retired)
                if ch + 1 < NCHUNK:
                    for ms in range(CHUNK // 128):
                        emit_x_stage(ch + 1, ms)

                sct = sctp.tile([64, CHUNK], FP, tag="sct", name=f"sct_{ch}")
                nc.vector.tensor_copy(sct[:], st[:])

                for ms in range(CHUNK // 128):
                    rows = slice(m0 + ms * 128, m0 + (ms + 1) * 128)
                    ps_s = ps64.tile([128, E], FP, tag="ps64",
                                     name=f"ps_s_{ch}_{ms}")
                    nc.tensor.transpose(
                        ps_s[:], sct[:, ms * 128:(ms + 1) * 128], ident[0:64, 0:64]
                    )
                    sc = epi.tile([128, E], FP, tag="sc")
                    nc.vector.scalar_tensor_tensor(
                        sc[:], in0=ps_s[:], scalar=1.0, in1=b2bc[:],
                        op0=OP.mult, op1=OP.add,
                    )
                    rowmax = epi.tile([128, 1], FP, tag="rowmax")
                    nc.vector.reduce_max(rowmax[:], sc[:], axis=mybir.AxisListType.X)
                    bias_t = epi.tile([128, 1], FP, tag="biast")
                    nc.vector.tensor_tensor(bias_t[:], rowmax[:], ninvt_bc[:],
                                            op=OP.mult)
                    eu = epi.tile([128, E], FP, tag="eu")
                    zsum = epi.tile([128, 1], FP, tag="zsum")
                    nc.scalar.activation(eu[:], sc[:], AF.Exp,
                                         bias=bias_t[:], scale=invt_bc[:],
                                         accum_out=zsum[:])
                    rz = epi.tile([128, 1], FP, tag="rz")
                    nc.vector.reciprocal(rz[:], zsum[:])
                    probs = epi.tile([128, E], FP, tag="probs")
                    nc.vector.tensor_scalar_mul(probs[:], eu[:], rz[:])
                    nc.sync.dma_start(attn_d[rows, :], probs[:])

                    max8 = epi.tile([128, 8], FP, tag="max8")
                    nc.vector.max(max8[:], probs[:])
                    idx8 = epi.tile([128, 8], mybir.dt.uint32, tag="idx8")
                    nc.vector.max_index(idx8[:], max8[:], probs[:])

                    idx_i = epi.tile([128, TOP_K], mybir.dt.int32, tag="idxi")
                    nc.vector.tensor_copy(idx_i[:], idx8[:, 0:TOP_K])
                    nc.sync.dma_start(idx_d[rows, :], idx_i[:])

                    idx_f = epi.tile([128, TOP_K], FP, tag="idxf")
                    nc.vector.tensor_copy(idx_f[:], idx8[:, 0:TOP_K])

                    s12 = epi.tile([128, 1], FP, tag="s12")
                    nc.vector.tensor_tensor(s12[:], max8[:, 0:1], max8[:, 1:2],
                                            op=OP.add)
                    r12 = epi.tile([128, 1], FP, tag="r12")
                    nc.vector.reciprocal(r12[:], s12[:])
                    wa = epi.tile([128, 1], FP, tag="wa")
                    nc.vector.tensor_tensor(wa[:], max8[:, 0:1], r12[:], op=OP.mult)
                    wb = epi.tile([128, 1], FP, tag="wb")
                    nc.vector.tensor_tensor(wb[:], max8[:, 1:2], r12[:], op=OP.mult)

                    t1 = epi.tile([128, E], FP, tag="t1")
                    nc.vector.tensor_scalar(
                        t1[:], iota_f[:], idx_f[:, 0:1], wa[:],
                        op0=OP.is_equal, op1=OP.mult,
                    )
                    t2 = epi.tile([128, E], FP, tag="t2")
                    nc.vector.tensor_scalar(
                        t2[:], iota_f[:], idx_f[:, 1:2], wb[:],
                        op0=OP.is_equal, op1=OP.mult,
                    )
                    nc.vector.tensor_tensor(t1[:], t1[:], t2[:], op=OP.add)
                    nc.sync.dma_start(mask_d[rows, :], t1[:])

    nc.compile()
    return nc


_NC_CACHE = None


def _get_program():
    global _NC_CACHE
    if _NC_CACHE is None:
        _NC_CACHE = build_program()
    return _NC_CACHE


def kernel(x, W1, b1, W2, b2, temperature, _trace=False):
    nc = _get_program()
    xs = np.ascontiguousarray(np.asarray(x, np.float32).reshape(TOKENS, D))
    in_maps = []
    for c in range(NCORES):
        in_maps.append({
            "x_shard": np.ascontiguousarray(xs[c * M_CORE:(c + 1) * M_CORE]),
            "W1": np.asarray(W1, np.float32),
            "b1": np.asarray(b1, np.float32),
            "W2": np.asarray(W2, np.float32),
            "b2": np.asarray(b2, np.float32),
            "temperature": np.asarray(temperature, np.float32),
        })
    kw = {}
    if _trace:
        kw = dict(trace=True)
    res = run_bass_kernel_spmd(nc, in_maps, core_ids=list(range(NCORES)), **kw)
    mask = np.concatenate([res.results[c]["mask_out"] for c in range(NCORES)], axis=0)
    attn = np.concatenate([res.results[c]["attn_out"] for c in range(NCORES)], axis=0)
    idx = np.concatenate([res.results[c]["idx_out"] for c in range(NCORES)], axis=0)

    usage = mask.astype(np.float64).sum(axis=0)           # [E]
    ideal = usage.sum() / E
    lbl = np.mean((usage - ideal) ** 2)
    ecl = np.mean(np.maximum(usage - CAPACITY, 0.0))
    loss = np.float32(lbl + ecl)

    mask = mask.reshape(B, S, E)
    attn = attn.reshape(B, S, E)
    idx = idx.reshape(B, S, TOP_K).astype(np.int32)
    if _trace:
        return (mask, loss, attn, idx), res
    return mask, loss, attn, idx


# revision 16
# speedup vs baseline: 1.1975x; 1.1975x over previous
"""AttentiveRouter (MoE routing) Trainium2 Bass kernel.

Full inputs in, full outputs out. Data-parallel over tokens (B*S=8192)
across 8 NeuronCores, router weights replicated. The load-balance
reduction over expert_usage is finished on host from the gathered
expert_mask (64-element math).

Numerics: matmuls run as fp16 hi/lo split triples (W@x = Wh@xh + Wh@xl
+ Wl@xh) accumulated into one fp32 PSUM chain, giving ~2^-22 effective
input precision (fp32-grade logits; top-k indices must match the fp32
reference exactly — min rank2/rank3 logit gap on this data is 3e-5).
The PE honors fp16 subnormal inputs (verified on HW), so the lo parts
need no scaling.

Per-core pipeline (1024 tokens, 2 chunks of 512):
  - x tile -> PE fp32 transpose -> split from PSUM into xhT/xlT fp16
  - W1 k-tiles DMA'd fp32 (gpsimd+scalar queues), split on ACT (cast)
    + DVE (subtract) into W1h/W1l fp16; emission interleaved with the
    x stage so the engine streams don't serialize the startup
  - mm1 (hT form) per nb block: one PSUM chain accumulating 3 passes
    x 16 k tiles; Gelu+b1 fused into the ACT drain of the PSUM (bias
    is per-partition in hT layout); hT split into fp16 halves
  - mm2 in scoresT form (W2h/W2l stationary, hT halves moving) into a
    [64,512] PSUM chain; PE-stream-pipelined one nb behind mm1 so the
    hT drain latency is hidden; scoresT is PE-transposed back to
    token-partition scores [128,64]; b2 added during the PSUM->SBUF
    copy
  - epilogue per 128-token tile: softmax (exp with scale=1/T and
    bias=-rowmax/T from the temperature input, accum-out Z), top-2 via
    DVE max/max_index, normalized weights, one-hot scatter via
    iota+is_equal
"""

import numpy as np

import concourse.bass as bass
import concourse.tile as tile
from concourse import bacc, masks, mybir
from concourse.bass_utils import run_bass_kernel_spmd

NCORES = 8
B, S, D, E = 4, 2048, 2048, 64
TOKENS = B * S              # 8192
M_CORE = TOKENS // NCORES   # 1024 tokens per core
CHUNK = 512                 # tokens per mm1 chunk
NCHUNK = M_CORE // CHUNK    # 2
KT = D // 128               # 16 k tiles
NBT = D // 128              # 16 n blocks
TOP_K = 2
CAPACITY = int(1.25 * S)    # 2560 (static, matches reference)

FP = mybir.dt.float32
HP = mybir.dt.float16


def build_program():
    nc = bacc.Bacc("TRN2", target_bir_lowering=False, debug=False)

    x_d = nc.dram_tensor("x_shard", [M_CORE, D], FP, kind="ExternalInput").ap()
    w1_d = nc.dram_tensor("W1", [D, D], FP, kind="ExternalInput").ap()
    b1_d = nc.dram_tensor("b1", [D], FP, kind="ExternalInput").ap()
    w2_d = nc.dram_tensor("W2", [D, E], FP, kind="ExternalInput").ap()
    b2_d = nc.dram_tensor("b2", [E], FP, kind="ExternalInput").ap()
    t_d = nc.dram_tensor("temperature", [1], FP, kind="ExternalInput").ap()

    mask_d = nc.dram_tensor("mask_out", [M_CORE, E], FP, kind="ExternalOutput").ap()
    attn_d = nc.dram_tensor("attn_out", [M_CORE, E], FP, kind="ExternalOutput").ap()
    idx_d = nc.dram_tensor(
        "idx_out", [M_CORE, TOP_K], mybir.dt.int32, kind="ExternalOutput"
    ).ap()

    AF = mybir.ActivationFunctionType
    OP = mybir.AluOpType

    with tile.TileContext(nc) as tc:
        with (
            tc.tile_pool(name="consts", bufs=1) as consts,
            tc.tile_pool(name="w1p", bufs=1) as w1p,
            tc.tile_pool(name="w2p", bufs=1) as w2p,
            tc.tile_pool(name="stage", bufs=3) as stage,
            tc.tile_pool(name="xtp", bufs=1) as xtp,
            tc.tile_pool(name="htp", bufs=2) as htp,
            tc.tile_pool(name="sct", bufs=1) as sctp,
            tc.tile_pool(name="epi", bufs=2) as epi,
            tc.tile_pool(name="ps512", bufs=4, space="PSUM") as ps512,
            tc.tile_pool(name="psT", bufs=2, space="PSUM") as psT,
            tc.tile_pool(name="ps64", bufs=2, space="PSUM") as ps64,
        ):
            # ---- constants ----
            ident = consts.tile([128, 128], FP, tag="ident")
            masks.make_identity(nc, ident[:])

            iota_f = consts.tile([128, E], FP, tag="iota")
            nc.gpsimd.iota(
                iota_f[:], pattern=[[1, E]], base=0, channel_multiplier=0,
                allow_small_or_imprecise_dtypes=True,
            )

            ones1 = consts.tile([1, 128], FP, tag="ones1")
            nc.vector.memset(ones1[:], 1.0)

            b2_sb = consts.tile([1, E], FP, tag="b2sb")
            nc.sync.dma_start(b2_sb[:], b2_d[None, :])

            b1t = consts.tile([128, NBT], FP, tag="b1t")
            nc.sync.dma_start(b1t[:], b1_d.rearrange("(a b) -> b a", b=128))

            t_sb = consts.tile([1, 1], FP, tag="tsb")
            nc.sync.dma_start(t_sb[:], t_d[None, :])
            invt1 = consts.tile([1, 1], FP, tag="invt1")
            nc.vector.reciprocal(invt1[:], t_sb[:])
            ps_invt = ps64.tile([128, 1], FP, tag="ps64", name="ps_invt")
            nc.tensor.matmul(ps_invt[:], lhsT=ones1[:], rhs=invt1[:])
            invt_bc = consts.tile([128, 1], FP, tag="invtbc")
            nc.scalar.copy(invt_bc[:], ps_invt[:])
            ninvt_bc = consts.tile([128, 1], FP, tag="ninvtbc")
            nc.scalar.mul(ninvt_bc[:], ps_invt[:], -1.0)

            ps_b2 = ps64.tile([128, E], FP, tag="ps64", name="ps_b2")
            nc.tensor.matmul(ps_b2[:], lhsT=ones1[:], rhs=b2_sb[:])
            b2bc = consts.tile([128, E], FP, tag="b2bc")
            nc.scalar.copy(b2bc[:], ps_b2[:])

            # ---- W2 load + split (small) ----
            w2s = stage.tile([128, NBT * E], FP, tag="stage", name="w2s")
            for nb in range(NBT):
                nc.sync.dma_start(
                    w2s[:, nb * E:(nb + 1) * E], w2_d[nb * 128:(nb + 1) * 128, :]
                )
            w2h = w2p.tile([128, NBT * E], HP, tag="w2h")
            nc.scalar.copy(w2h[:], w2s[:])
            w2l = w2p.tile([128, NBT * E], HP, tag="w2l")
            nc.vector.scalar_tensor_tensor(
                w2l[:], in0=w2h[:], scalar=-1.0, in1=w2s[:],
                op0=OP.mult, op1=OP.add,
            )

            w1h = [None] * KT
            w1l = [None] * KT
            xh = [
                xtp.tile([128, CHUNK], HP, tag=f"xh_{k}", name=f"xh_{k}")
                for k in range(KT)
            ]
            xl = [
                xtp.tile([128, CHUNK], HP, tag=f"xl_{k}", name=f"xl_{k}")
                for k in range(KT)
            ]

            def emit_w1_split(k):
                ws = stage.tile([128, D], FP, tag="stage", name=f"w1s_{k}")
                eng = nc.gpsimd if (k % 2 == 0) else nc.scalar
                eng.dma_start(ws[:], w1_d[k * 128:(k + 1) * 128, :])
                wh = w1p.tile([128, D], HP, tag=f"w1h_{k}", name=f"w1h_{k}")
                nc.scalar.copy(wh[:], ws[:])
                wl = w1p.tile([128, D], HP, tag=f"w1l_{k}", name=f"w1l_{k}")
                nc.vector.scalar_tensor_tensor(
                    wl[:], in0=wh[:], scalar=-1.0, in1=ws[:],
                    op0=OP.mult, op1=OP.add,
                )
                w1h[k] = wh
                w1l[k] = wl

            def emit_x_stage(ch, ms):
                m0 = ch * CHUNK
                xs = stage.tile([128, D], FP, tag="stage", name=f"xs_{ch}_{ms}")
                nc.sync.dma_start(xs[:], x_d[m0 + ms * 128:m0 + (ms + 1) * 128, :])
                cols = slice(ms * 128, (ms + 1) * 128)
                for k in range(KT):
                    pt = ps512.tile([128, 128], FP, tag="ps512",
                                    name=f"pt_{ch}_{ms}_{k}")
                    nc.tensor.transpose(pt[:], xs[:, k * 128:(k + 1) * 128],
                                        ident[:])
                    nc.scalar.copy(xh[k][:, cols], pt[:])
                    nc.vector.scalar_tensor_tensor(
                        xl[k][:, cols], in0=xh[k][:, cols], scalar=-1.0,
                        in1=pt[:], op0=OP.mult, op1=OP.add,
                    )

            def emit_mm2(st, nb, hth, htl):
                ec = slice(nb * E, (nb + 1) * E)
                nc.tensor.matmul(st[:], lhsT=w2h[:, ec], rhs=hth[:],
                                 start=(nb == 0), stop=False)
                nc.tensor.matmul(st[:], lhsT=w2h[:, ec], rhs=htl[:],
                                 start=False, stop=False)
                nc.tensor.matmul(st[:], lhsT=w2l[:, ec], rhs=hth[:],
                                 start=False, stop=(nb == NBT - 1))

            # interleave W1 split with chunk-0 x staging so the ACT/DVE
            # streams alternate between them during the DMA-bound startup
            for k in range(KT):
                emit_w1_split(k)
                if k % 4 == 3:
                    emit_x_stage(0, k // 4)

            def emit_ht_split(ch, nb, acc):
                ht32 = htp.tile([128, CHUNK], FP, tag="ht32",
                                name=f"ht32_{ch}_{nb}")
                nc.scalar.activation(ht32[:], acc[:], AF.Gelu,
                                     bias=b1t[:, nb:nb + 1])
                hth = htp.tile([128, CHUNK], HP, tag="hth",
                               name=f"hth_{ch}_{nb}")
                nc.scalar.copy(hth[:], ht32[:])
                htl = htp.tile([128, CHUNK], HP, tag="htl",
                               name=f"htl_{ch}_{nb}")
                nc.vector.scalar_tensor_tensor(
                    htl[:], in0=hth[:], scalar=-1.0, in1=ht32[:],
                    op0=OP.mult, op1=OP.add,
                )
                return hth, htl

            for ch in range(NCHUNK):
                m0 = ch * CHUNK

                st = psT.tile([64, CHUNK], FP, tag="psT", name=f"st_{ch}")
                pending = None
                nb_start = 0
                if ch == 0:
                    # startup: hi sweeps first — W1h tiles (ACT cast) land
                    # well before W1l (DVE subtract) during the DMA-paced
                    # W1 split
                    nb_start = 2
                    for nb in range(2):
                        acc = ps512.tile([128, CHUNK], FP, tag="ps512",
                                         name=f"acc_{ch}_{nb}")
                        nbc = slice(nb * 128, (nb + 1) * 128)
                        for k in range(KT):
                            nc.tensor.matmul(acc[:], lhsT=w1h[k][:, nbc],
                                             rhs=xh[k][:],
                                             start=(k == 0), stop=False)
                        for k in range(KT):
                            nc.tensor.matmul(acc[:], lhsT=w1h[k][:, nbc],
                                             rhs=xl[k][:],
                                             start=False, stop=False)
                            nc.tensor.matmul(acc[:], lhsT=w1l[k][:, nbc],
                                             rhs=xh[k][:],
                                             start=False, stop=(k == KT - 1))
                        if pending is not None:
                            emit_mm2(st, *pending)
                        hth, htl = emit_ht_split(ch, nb, acc)
                        pending = (nb, hth, htl)

                for nb in range(nb_start, NBT):
                    acc = ps512.tile([128, CHUNK], FP, tag="ps512",
                                     name=f"acc_{ch}_{nb}")
                    nbc = slice(nb * 128, (nb + 1) * 128)
                    for k in range(KT):
                        nc.tensor.matmul(acc[:], lhsT=w1h[k][:, nbc],
                                         rhs=xh[k][:],
                                         start=(k == 0), stop=False)
                        nc.tensor.matmul(acc[:], lhsT=w1h[k][:, nbc],
                                         rhs=xl[k][:],
                                         start=False, stop=False)
                        nc.tensor.matmul(acc[:], lhsT=w1l[k][:, nbc],
                                         rhs=xh[k][:],
                                         start=False, stop=(k == KT - 1))
                    # PE pipeline: previous nb's mm2 lands after this nb's
                    # mm1 so its hT drain latency is hidden
                    if pending is not None:
                        emit_mm2(st, *pending)
                    hth, htl = emit_ht_split(ch, nb, acc)
                    pending = (nb, hth, htl)
                emit_mm2(st, *pending)

                # stage the NEXT chunk's x before this chunk's epilogue so
                # its PE transposes run while the DVE handles the scoresT
                # copy (the xh/xl WAR hazard is already clear: last mm1
                # read of this chunk just retired)
                if ch + 1 < NCHUNK:
                    for ms in range(CHUNK // 128):
                        emit_x_stage(ch + 1, ms)

                sct = sctp.tile([64, CHUNK], FP, tag="sct", name=f"sct_{ch}")
                nc.vector.tensor_copy(sct[:], st[:])

                for ms in range(CHUNK // 128):
                    rows = slice(m0 + ms * 128, m0 + (ms + 1) * 128)
                    ps_s = ps64.tile([128, E], FP, tag="ps64",
                                     name=f"ps_s_{ch}_{ms}")
                    nc.tensor.transpose(
                        ps_s[:], sct[:, ms * 128:(ms + 1) * 128], ident[0:64, 0:64]
                    )
                    sc = epi.tile([128, E], FP, tag="sc")
                    nc.vector.scalar_tensor_tensor(
                        sc[:], in0=ps_s[:], scalar=1.0, in1=b2bc[:],
                        op0=OP.mult, op1=OP.add,
                    )
                    rowmax = epi.tile([128, 1], FP, tag="rowmax")
                    nc.vector.reduce_max(rowmax[:], sc[:], axis=mybir.AxisListType.X)
                    bias_t = epi.tile([128, 1], FP, tag="biast")
                    nc.vector.tensor_tensor(bias_t[:], rowmax[:], ninvt_bc[:],
                                            op=OP.mult)
                    eu = epi.tile([128, E], FP, tag="eu")
                    zsum = epi.tile([128, 1], FP, tag="zsum")
                    nc.scalar.activation(eu[:], sc[:], AF.Exp,
                                         bias=bias_t[:], scale=invt_bc[:],
                                         accum_out=zsum[:])
                    rz = epi.tile([128, 1], FP, tag="rz")
                    nc.vector.reciprocal(rz[:], zsum[:])
                    probs = epi.tile([128, E], FP, tag="probs")
                    nc.vector.tensor_scalar_mul(probs[:], eu[:], rz[:])
                    nc.sync.dma_start(attn_d[rows, :], probs[:])

                    max8 = epi.tile([128, 8], FP, tag="max8")
                    nc.vector.max(max8[:], probs[:])
                    idx8 = epi.tile([128, 8], mybir.dt.uint32, tag="idx8")
                    nc.vector.max_index(idx8[:], max8[:], probs[:])

                    idx_i = epi.tile([128, TOP_K], mybir.dt.int32, tag="idxi")
                    nc.vector.tensor_copy(idx_i[:], idx8[:, 0:TOP_K])
                    nc.sync.dma_start(idx_d[rows, :], idx_i[:])

                    idx_f = epi.tile([128, TOP_K], FP, tag="idxf")
                    nc.vector.tensor_copy(idx_f[:], idx8[:, 0:TOP_K])

                    s12 = epi.tile([128, 1], FP, tag="s12")
                    nc.vector.tensor_tensor(s12[:], max8[:, 0:1], max8[:, 1:2],
                                            op=OP.add)
                    r12 = epi.tile([128, 1], FP, tag="r12")
                    nc.vector.reciprocal(r12[:], s12[:])
                    wa = epi.tile([128, 1], FP, tag="wa")
                    nc.vector.tensor_tensor(wa[:], max8[:, 0:1], r12[:], op=OP.mult)
                    wb = epi.tile([128, 1], FP, tag="wb")
                    nc.vector.tensor_tensor(wb[:], max8[:, 1:2], r12[:], op=OP.mult)

                    t1 = epi.tile([128, E], FP, tag="t1")
                    nc.vector.tensor_scalar(
                        t1[:], iota_f[:], idx_f[:, 0:1], wa[:],
                        op0=OP.is_equal, op1=OP.mult,
                    )
                    t2 = epi.tile([128, E], FP, tag="t2")
                    nc.vector.tensor_scalar(
                        t2[:], iota_f[:], idx_f[:, 1:2], wb[:],
                        op0=OP.is_equal, op1=OP.mult,
                    )
                    nc.vector.tensor_tensor(t1[:], t1[:], t2[:], op=OP.add)
                    nc.sync.dma_start(mask_d[rows, :], t1[:])

    nc.compile()
    return nc


_NC_CACHE = None


def _get_program():
    global _NC_CACHE
    if _NC_CACHE is None:
        _NC_CACHE = build_program()
    return _NC_CACHE


def kernel(x, W1, b1, W2, b2, temperature, _trace=False):
    nc = _get_program()
    xs = np.ascontiguousarray(np.asarray(x, np.float32).reshape(TOKENS, D))
    in_maps = []
    for c in range(NCORES):
        in_maps.append({
            "x_shard": np.ascontiguousarray(xs[c * M_CORE:(c + 1) * M_CORE]),
            "W1": np.asarray(W1, np.float32),
            "b1": np.asarray(b1, np.float32),
            "W2": np.asarray(W2, np.float32),
            "b2": np.asarray(b2, np.float32),
            "temperature": np.asarray(temperature, np.float32),
        })
    kw = {}
    if _trace:
        kw = dict(trace=True)
    res = run_bass_kernel_spmd(nc, in_maps, core_ids=list(range(NCORES)), **kw)
    mask = np.concatenate([res.results[c]["mask_out"] for c in range(NCORES)], axis=0)
    attn = np.concatenate([res.results[c]["attn_out"] for c in range(NCORES)], axis=0)
    idx = np.concatenate([res.results[c]["idx_out"] for c in range(NCORES)], axis=0)

    usage = mask.astype(np.float64).sum(axis=0)           # [E]
    ideal = usage.sum() / E
    lbl = np.mean((usage - ideal) ** 2)
    ecl = np.mean(np.maximum(usage - CAPACITY, 0.0))
    loss = np.float32(lbl + ecl)

    mask = mask.reshape(B, S, E)
    attn = attn.reshape(B, S, E)
    idx = idx.reshape(B, S, TOP_K).astype(np.int32)
    if _trace:
        return (mask, loss, attn, idx), res
    return mask, loss, attn, idx
